# revision 6
# baseline (speedup 1.0000x reference)
"""DecoderBertModel Trainium2 kernel.

Sharding: DP-4 over batch x TP-2 (heads/FFN/vocab split) across 8 cores.
Core c = 2*b + r handles batch element b, TP rank r (6 heads, FFN-1536,
vocab half). Pairwise AllReduce after self-attn out-proj, cross-attn
out-proj, and FFN second matmul. Host does the embedding gather +
embedding LayerNorm + mask expansion + final assembly.

On-device dataflow: all activations feature-major [feature_part, token]:
matmuls chain with zero on-device transposes. Scores are computed
transposed (S_T[k,q]); softmax normalization sums come from ones-vector
matmuls over the k-partition axis; per-token scalars are broadcast
across partitions with K=1 matmuls using bf16 hi/lo splitting for
precision. Matmul operands are bf16 (fp32 PSUM accumulation); the
residual stream, LayerNorm, softmax and logits stay fp32.
"""
import math
import numpy as np

H, NH, DH, FF, L, V, MAXP = 768, 12, 64, 3072, 6, 30522, 512
B, S, SE = 4, 512, 512
EPS = 1e-12
N_CORES = 8
TP = 2
HC = H // 128           # 6 feature chunks
LH = NH // TP           # 6 local heads
LF = LH * DH            # 384 local attn features
LFF = FF // TP          # 1536 local ffn features
FFC = LFF // 128        # 12 ffn chunks
VSH = V // TP           # 15261 local vocab
VPAD = 15360            # padded to 30*512
NVC = VPAD // 512       # 30 vocab chunks
TC = S // 128           # 4 token tiles
KC = S // 128           # 4 key chunks

_RUNNER_CACHE = {}


# ----------------------------------------------------------------------------
# device kernel builder
# ----------------------------------------------------------------------------

def build_nc(n_layers=L):
    import concourse.bacc as bacc
    import concourse.mybir as mybir
    import concourse.tile as tile
    from concourse.bass import ts

    F32 = mybir.dt.float32
    BF16 = mybir.dt.bfloat16
    AF = mybir.ActivationFunctionType
    ALU = mybir.AluOpType

    nc = bacc.Bacc("TRN2", target_bir_lowering=False, debug=False,
                   num_devices=N_CORES)

    def din(name, shape, dt=F32):
        return nc.dram_tensor(name, shape, dt, kind="ExternalInput")

    x0T = din("x0T", [128, HC, S])
    x0Tb = din("x0Tb", [128, HC, S], BF16)
    encTb = din("encTb", [128, HC, SE], BF16)
    maskT = din("maskT", [128, KC, S])          # self-attn additive [k_in, kc, q]
    encmask = din("encmask", [128, KC])         # cross additive per key
    wqkv = [din(f"wqkv_{l}_{a}", [128, HC, 3 * LF], BF16)
            for l in range(n_layers) for a in range(2)]
    wo = [din(f"wo_{l}_{a}", [128, LF // 128, H], BF16)
          for l in range(n_layers) for a in range(2)]
    # bvec cols: 0:3 bq*0.125, 3:6 bk, 6:12 bo, 12:18 ln_g, 18:24 ln_b
    bvec = [din(f"bvec_{l}_{a}", [128, 24]) for l in range(n_layers) for a in range(2)]
    bv_attn = [din(f"bv_{l}_{a}", [1, LF], BF16) for l in range(n_layers) for a in range(2)]
    w1 = [din(f"w1_{l}", [128, HC, LFF], BF16) for l in range(n_layers)]
    w2 = [din(f"w2_{l}", [128, FFC, H], BF16) for l in range(n_layers)]
    # bffn cols: 0:12 b1, 12:18 b2, 18:24 ln_g, 24:30 ln_b
    bffn = [din(f"bffn_{l}", [128, FFC + 3 * HC]) for l in range(n_layers)]
    cls_w = din("cls_w", [128, HC, H], BF16)
    cls_bvec = din("cls_bvec", [128, 3 * HC])   # 0:6 cls_b, 6:12 ln_g, 12:18 ln_b
    emb = din("emb", [128, HC, VPAD], BF16)
    bias_rep = din("bias_rep", [128, VPAD])
    logits = nc.dram_tensor("logits", [S, VPAD], F32, kind="ExternalOutput")

    rg = [[0, 1], [2, 3], [4, 5], [6, 7]]

    with tile.TileContext(nc) as tc:
        with (
            tc.tile_pool(name="singles", bufs=1) as singles,
            tc.tile_pool(name="wat", bufs=4) as wat,
            tc.tile_pool(name="wff", bufs=2) as wff,
            tc.tile_pool(name="wsm", bufs=2) as wsm,
            tc.tile_pool(name="res", bufs=2) as res,
            tc.tile_pool(name="act", bufs=2) as actp,
            tc.tile_pool(name="act1", bufs=1) as act1,
            tc.tile_pool(name="pt", bufs=2) as ptp,
            tc.tile_pool(name="dram", bufs=2, space="DRAM") as dram,
            tc.tile_pool(name="mm", bufs=4, space="PSUM") as pmm,
            tc.tile_pool(name="stat", bufs=2, space="PSUM") as pstat,
            tc.tile_pool(name="aux", bufs=2, space="PSUM") as paux,
        ):
            # --- static tiles
            encT_sb = singles.tile([128, HC, SE], BF16, tag="encT")
            nc.sync.dma_start(encT_sb[:], encTb[:])
            maskT_sb = singles.tile([128, KC, S], F32, tag="maskT")
            nc.sync.dma_start(maskT_sb[:], maskT[:])
            encm_sb = singles.tile([128, KC], F32, tag="encm")
            nc.sync.dma_start(encm_sb[:], encmask[:])
            ones_col = singles.tile([128, 1], BF16, tag="ones_col")
            nc.vector.memset(ones_col[:], 1.0)
            ones_row = singles.tile([1, 128], BF16, tag="ones_row")
            nc.vector.memset(ones_row[:], 1.0)
            eps_t = singles.tile([1, 1], F32, tag="eps")
            nc.vector.memset(eps_t[:], EPS)

            def bcast_hilo(vec_f32, npart, accum_into=None):
                """Broadcast [1,S] fp32 -> psum [npart,S] via bf16 hi/lo MMs."""
                hi = actp.tile([1, S], BF16, tag="bc_hi")
                lo = actp.tile([1, S], BF16, tag="bc_lo")
                t32 = actp.tile([1, S], F32, tag="bc_t32")
                nc.vector.tensor_copy(hi[:], vec_f32)
                nc.vector.tensor_tensor(t32[:], vec_f32, hi[:], ALU.subtract)
                nc.vector.tensor_copy(lo[:], t32[:])
                if accum_into is None:
                    ps = paux.tile([npart, S], F32, tag="aux", name="bc_ps")
                else:
                    ps = accum_into
                nc.tensor.matmul(ps[:], lhsT=ones_row[:, :npart], rhs=hi[:],
                                 start=True, stop=False)
                nc.tensor.matmul(ps[:], lhsT=ones_row[:, :npart], rhs=lo[:],
                                 start=False, stop=True)
                return ps

            def layer_norm(z, zb_out, g_ap, b_ap):
                """In-place LN of fp32 z [128, HC, S] (feature-major), also
                writes bf16 shadow zb_out. g_ap/b_ap: [128, HC] slices."""
                ps_s1 = pstat.tile([1, S], F32, tag="stat")
                ps_s2 = pstat.tile([1, S], F32, tag="stat")
                for c in range(HC):
                    zb = actp.tile([128, S], BF16, tag="ln_zb")
                    nc.scalar.activation(zb[:], z[:, c, :], AF.Identity)
                    sq = actp.tile([128, S], BF16, tag="ln_sq")
                    nc.vector.tensor_mul(sq[:], zb[:], zb[:])
                    nc.tensor.matmul(ps_s1[:], lhsT=ones_col[:], rhs=zb[:],
                                     start=(c == 0), stop=(c == HC - 1))
                    nc.tensor.matmul(ps_s2[:], lhsT=ones_col[:], rhs=sq[:],
                                     start=(c == 0), stop=(c == HC - 1))
                mean = act1.tile([1, S], F32, tag="ln_mean")
                var = act1.tile([1, S], F32, tag="ln_var")
                rstd = act1.tile([1, S], F32, tag="ln_rstd")
                nc.scalar.mul(mean[:], ps_s1[:], 1.0 / H)
                nc.scalar.mul(var[:], ps_s2[:], 1.0 / H)
                nc.vector.tensor_mul(rstd[:], mean[:], mean[:])
                nc.vector.tensor_sub(var[:], var[:], rstd[:])
                nc.scalar.activation(rstd[:], var[:], AF.Sqrt,
                                     bias=eps_t[:], scale=1.0)
                nc.vector.reciprocal(rstd[:], rstd[:])
                ps_mb = bcast_hilo(mean[:], 128)
                ps_rb = bcast_hilo(rstd[:], 128)
                tmp = actp.tile([128, S], F32, tag="ln_tmp")
                for c in range(HC):
                    nc.vector.tensor_tensor(tmp[:], z[:, c, :], ps_mb[:], ALU.subtract)
                    nc.vector.tensor_tensor(tmp[:], tmp[:], ps_rb[:], ALU.mult)
                    nc.vector.tensor_scalar(z[:, c, :], tmp[:],
                                            g_ap[:, c, None], b_ap[:, c, None],
                                            op0=ALU.mult, op1=ALU.add)
                    nc.scalar.activation(zb_out[:, c, :], z[:, c, :], AF.Identity)

            def all_reduce(part_sb, out_sb):
                b_in = dram.tile([128, HC, S], F32, tag="ar_in")
                b_out = dram.tile([128, HC, S], F32, tag="ar_out")
                nc.sync.dma_start(b_in[:], part_sb[:])
                nc.gpsimd.collective_compute(
                    "AllReduce", ALU.add,
                    ins=[b_in[:].opt()], outs=[b_out[:].opt()],
                    replica_groups=rg,
                )
                nc.sync.dma_start(out_sb[:], b_out[:])

            def attn_block(qin, qin_b, kvin_b, kv_len, wqkv_t, wo_t, bvec_t,
                           bv_t, is_self):
                """qin: fp32 resid [128,HC,S]; qin_b/kvin_b: bf16 shadows.
                Returns (new fp32 resid tile, new bf16 shadow tile)."""
                kcn = kv_len // 128
                wq_sb = wat.tile([128, HC, LF], BF16, tag="wat")
                nc.sync.dma_start(wq_sb[:], wqkv_t[:, :, 0:LF])
                wk_sb = wat.tile([128, HC, LF], BF16, tag="wat")
                nc.sync.dma_start(wk_sb[:], wqkv_t[:, :, LF:2 * LF])
                wv_sb = wat.tile([128, HC, LF], BF16, tag="wat")
                nc.sync.dma_start(wv_sb[:], wqkv_t[:, :, 2 * LF:3 * LF])
                wo_sb = wat.tile([128, LF // 128, H], BF16, tag="wat")
                nc.sync.dma_start(wo_sb[:], wo_t[:])
                bv_sb = wsm.tile([128, 24], F32, tag="bvec")
                nc.sync.dma_start(bv_sb[:], bvec_t[:])
                bvv_sb = wsm.tile([1, LF], BF16, tag="bv_attn")
                nc.sync.dma_start(bvv_sb[:], bv_t[:])

                qT = act1.tile([128, LF // 128, S], BF16, tag="qT")
                kT = act1.tile([128, LF // 128, SE], BF16, tag="kT")
                vtm = act1.tile([128, KC, LF], BF16, tag="vtm")
                for f in range(LF // 128):
                    pq = pmm.tile([128, S], F32, tag="mm")
                    for c in range(HC):
                        nc.tensor.matmul(pq[:], lhsT=wq_sb[:, c, ts(f, 128)],
                                         rhs=qin_b[:, c, :],
                                         start=(c == 0), stop=(c == HC - 1))
                    nc.scalar.activation(qT[:, f, :], pq[:], AF.Identity,
                                         bias=bv_sb[:, f, None], scale=0.125)
                for f in range(LF // 128):
                    pk = pmm.tile([128, kv_len], F32, tag="mm")
                    for c in range(HC):
                        nc.tensor.matmul(pk[:], lhsT=wk_sb[:, c, ts(f, 128)],
                                         rhs=kvin_b[:, c, :],
                                         start=(c == 0), stop=(c == HC - 1))
                    nc.scalar.activation(kT[:, f, :kv_len], pk[:], AF.Identity,
                                         bias=bv_sb[:, 3 + f, None], scale=1.0)
                for t in range(kcn):
                    pv = pmm.tile([128, LF], F32, tag="mm")
                    for c in range(HC):
                        nc.tensor.matmul(pv[:], lhsT=kvin_b[:, c, ts(t, 128)],
                                         rhs=wv_sb[:, c, :],
                                         start=(c == 0), stop=False)
                    nc.tensor.matmul(pv[:], lhsT=ones_row[:], rhs=bvv_sb[:],
                                     start=False, stop=True)
                    nc.scalar.activation(vtm[:, t, :], pv[:], AF.Identity)

                ctxT = act1.tile([128, LF // 128, S], BF16, tag="ctxT")
                for h in range(LH):
                    fchunk, off = h // 2, (h % 2) * DH
                    pT = ptp.tile([128, kcn, S], BF16, tag="pT")
                    for kc in range(kcn):
                        psc = pmm.tile([128, S], F32, tag="mm")
                        nc.tensor.matmul(
                            psc[:],
                            lhsT=kT[off:off + DH, fchunk, ts(kc, 128)],
                            rhs=qT[off:off + DH, fchunk, :],
                            start=True, stop=True)
                        if is_self:
                            stmp = actp.tile([128, S], F32, tag="stmp")
                            nc.vector.tensor_add(stmp[:], psc[:],
                                                 maskT_sb[:, kc, :])
                            nc.scalar.activation(pT[:, kc, :], stmp[:], AF.Exp)
                        else:
                            nc.scalar.activation(pT[:, kc, :], psc[:], AF.Exp,
                                                 bias=encm_sb[:, kc, None],
                                                 scale=1.0)
                    ps_sum = pstat.tile([1, S], F32, tag="stat")
                    for kc in range(kcn):
                        nc.tensor.matmul(ps_sum[:], lhsT=ones_col[:],
                                         rhs=pT[:, kc, :],
                                         start=(kc == 0), stop=(kc == kcn - 1))
                    rcp = act1.tile([1, S], F32, tag="rcp")
                    nc.vector.reciprocal(rcp[:], ps_sum[:])
                    ps_rcpb = bcast_hilo(rcp[:], DH)
                    rcpb_sb = actp.tile([DH, S], F32, tag="rcpb_sb")
                    nc.scalar.activation(rcpb_sb[:], ps_rcpb[:], AF.Identity)
                    ps_ctx = paux.tile([DH, S], F32, tag="aux")
                    for kc in range(kcn):
                        nc.tensor.matmul(ps_ctx[:],
                                         lhsT=vtm[:, kc, ts(h, DH)],
                                         rhs=pT[:, kc, :],
                                         start=(kc == 0), stop=(kc == kcn - 1))
                    nc.vector.tensor_mul(ctxT[off:off + DH, fchunk, :],
                                         ps_ctx[:], rcpb_sb[:])

                zpart = act1.tile([128, HC, S], F32, tag="zpart")
                for c in range(HC):
                    po = pmm.tile([128, S], F32, tag="mm")
                    for f in range(LF // 128):
                        nc.tensor.matmul(po[:], lhsT=wo_sb[:, f, ts(c, 128)],
                                         rhs=ctxT[:, f, :],
                                         start=(f == 0), stop=(f == LF // 128 - 1))
                    nc.scalar.activation(zpart[:, c, :], po[:], AF.Identity)
                z = res.tile([128, HC, S], F32, tag="res")
                all_reduce(zpart, z)
                for c in range(HC):
                    nc.vector.tensor_scalar(z[:, c, :], z[:, c, :],
                                            bv_sb[:, 6 + c, None], None,
                                            op0=ALU.add)
                    nc.vector.tensor_add(z[:, c, :], z[:, c, :], qin[:, c, :])
                zb = res.tile([128, HC, S], BF16, tag="resb")
                layer_norm(z, zb, bv_sb[:, 12:18], bv_sb[:, 18:24])
                return z, zb

            def ffn_block(a, a_b, w1_t, w2_t, bffn_t):
                w1h = []
                for i in range(2):
                    w1_sb = wff.tile([128, HC, LFF // 2], BF16, tag="wff",
                                     name=f"w1_sb{i}")
                    nc.sync.dma_start(
                        w1_sb[:], w1_t[:, :, i * (LFF // 2):(i + 1) * (LFF // 2)])
                    w1h.append(w1_sb)
                bf_sb = wsm.tile([128, FFC + 3 * HC], F32, tag="bffn")
                nc.sync.dma_start(bf_sb[:], bffn_t[:])
                hT = act1.tile([128, FFC, S], BF16, tag="hT")
                for fc in range(FFC):
                    w1_sb, fo = (w1h[0], fc) if fc < FFC // 2 else (w1h[1], fc - FFC // 2)
                    ph = pmm.tile([128, S], F32, tag="mm")
                    for c in range(HC):
                        nc.tensor.matmul(ph[:], lhsT=w1_sb[:, c, ts(fo, 128)],
                                         rhs=a_b[:, c, :],
                                         start=(c == 0), stop=(c == HC - 1))
                    nc.scalar.activation(hT[:, fc, :], ph[:], AF.Gelu,
                                         bias=bf_sb[:, fc, None], scale=1.0)
                w2h = []
                for i in range(2):
                    w2_sb = wff.tile([128, FFC // 2, H], BF16, tag="wff",
                                     name=f"w2_sb{i}")
                    nc.sync.dma_start(
                        w2_sb[:], w2_t[:, i * (FFC // 2):(i + 1) * (FFC // 2), :])
                    w2h.append(w2_sb)
                zpart = act1.tile([128, HC, S], F32, tag="zpart")
                for c in range(HC):
                    pz = pmm.tile([128, S], F32, tag="mm")
                    for fc in range(FFC):
                        w2_sb, fo = (w2h[0], fc) if fc < FFC // 2 else (w2h[1], fc - FFC // 2)
                        nc.tensor.matmul(pz[:], lhsT=w2_sb[:, fo, ts(c, 128)],
                                         rhs=hT[:, fc, :],
                                         start=(fc == 0), stop=(fc == FFC - 1))
                    nc.scalar.activation(zpart[:, c, :], pz[:], AF.Identity)
                z = res.tile([128, HC, S], F32, tag="res")
                all_reduce(zpart, z)
                for c in range(HC):
                    nc.vector.tensor_scalar(z[:, c, :], z[:, c, :],
                                            bf_sb[:, FFC + c, None], None,
                                            op0=ALU.add)
                    nc.vector.tensor_add(z[:, c, :], z[:, c, :], a[:, c, :])
                zb = res.tile([128, HC, S], BF16, tag="resb")
                layer_norm(z, zb, bf_sb[:, FFC + HC:FFC + 2 * HC],
                           bf_sb[:, FFC + 2 * HC:FFC + 3 * HC])
                return z, zb

            # ---- load embeddings into residual stream
            x = res.tile([128, HC, S], F32, tag="res")
            nc.sync.dma_start(x[:], x0T[:])
            xb = res.tile([128, HC, S], BF16, tag="resb")
            nc.sync.dma_start(xb[:], x0Tb[:])

            # ---- transformer body
            for l in range(n_layers):
                a, ab = attn_block(x, xb, xb, S, wqkv[2 * l], wo[2 * l],
                                   bvec[2 * l], bv_attn[2 * l], is_self=True)
                a2, a2b = attn_block(a, ab, encT_sb, SE, wqkv[2 * l + 1],
                                     wo[2 * l + 1], bvec[2 * l + 1],
                                     bv_attn[2 * l + 1], is_self=False)
                x, xb = ffn_block(a2, a2b, w1[l], w2[l], bffn[l])

            # ---- classifier head: t = LN(gelu(x @ cls_w.T + cls_b))
            cw_sb = wff.tile([128, HC, H], BF16, tag="wff")
            nc.sync.dma_start(cw_sb[:], cls_w[:])
            cb_sb = wsm.tile([128, 3 * HC], F32, tag="bffn")
            nc.sync.dma_start(cb_sb[:], cls_bvec[:])
            g = res.tile([128, HC, S], F32, tag="res")
            for c in range(HC):
                pg = pmm.tile([128, S], F32, tag="mm")
                for c2 in range(HC):
                    nc.tensor.matmul(pg[:], lhsT=cw_sb[:, c2, ts(c, 128)],
                                     rhs=xb[:, c2, :],
                                     start=(c2 == 0), stop=(c2 == HC - 1))
                nc.scalar.activation(g[:, c, :], pg[:], AF.Gelu,
                                     bias=cb_sb[:, c, None], scale=1.0)
            tb = res.tile([128, HC, S], BF16, tag="resb")
            layer_norm(g, tb, cb_sb[:, HC:2 * HC], cb_sb[:, 2 * HC:3 * HC])

            # ---- vocab projection: logits[t, v] = t.T @ emb + bias
            for vc in range(NVC):
                ech = wff.tile([128, HC, 512], BF16, tag="wff")
                nc.sync.dma_start(ech[:], emb[:, :, ts(vc, 512)])
                bch = wsm.tile([128, 512], F32, tag="bias_ch")
                nc.sync.dma_start(bch[:], bias_rep[:, ts(vc, 512)])
                for t in range(TC):
                    pl = pmm.tile([128, 512], F32, tag="mm")
                    for c in range(HC):
                        nc.tensor.matmul(pl[:], lhsT=tb[:, c, ts(t, 128)],
                                         rhs=ech[:, c, :],
                                         start=(c == 0), stop=(c == HC - 1))
                    lsb = actp.tile([128, 512], F32, tag="lsb")
                    nc.vector.tensor_add(lsb[:], pl[:], bch[:])
                    nc.sync.dma_start(logits[ts(t, 128), ts(vc, 512)], lsb[:])

    nc.finalize()
    return nc


# ----------------------------------------------------------------------------
# host-side preprocessing
# ----------------------------------------------------------------------------

def _np(x):
    return np.asarray(x)


def _bf16(x):
    import ml_dtypes
    return np.asarray(x, dtype=np.float32).astype(ml_dtypes.bfloat16)


def _ln_host(x, g, b):
    m = x.mean(axis=-1, keepdims=True)
    v = ((x - m) ** 2).mean(axis=-1, keepdims=True)
    return (x - m) / np.sqrt(v + EPS) * g + b


def _fmaj(x):
    """[S, H] -> [128, H//128, S] feature-major chunks."""
    s, h = x.shape
    return np.ascontiguousarray(
        x.T.reshape(h // 128, 128, s).transpose(1, 0, 2)).astype(np.float32)


def _wT(w):
    """torch Linear weight [out, in] -> [128, in//128, out] lhsT layout."""
    win = np.ascontiguousarray(np.asarray(w, dtype=np.float32).T)  # [in, out]
    i, o = win.shape
    return np.ascontiguousarray(
        win.reshape(i // 128, 128, o).transpose(1, 0, 2))


def _pvec(v):
    """[F] -> [128, F//128] per-partition layout."""
    v = np.asarray(v, dtype=np.float32)
    f = v.shape[0]
    return np.ascontiguousarray(v.reshape(f // 128, 128).T)


def host_inputs(input_ids, encoder_outs, answer_mask, encoder_mask, params,
                n_layers=L):
    p = params
    input_ids = _np(input_ids)
    encoder_outs = np.asarray(_np(encoder_outs), dtype=np.float32)
    answer_mask = np.asarray(_np(answer_mask), dtype=np.float32)
    encoder_mask = np.asarray(_np(encoder_mask), dtype=np.float32)

    word_emb = np.asarray(_np(p["word_emb"]), dtype=np.float32)
    pos_emb = np.asarray(_np(p["pos_emb"]), dtype=np.float32)
    emb = word_emb[input_ids] + pos_emb[:S][None]
    x0 = _ln_host(emb.astype(np.float64),
                  np.asarray(_np(p["emb_ln_g"]), dtype=np.float64),
                  np.asarray(_np(p["emb_ln_b"]), dtype=np.float64)
                  ).astype(np.float32)

    sub = np.triu(np.ones((S, S), np.float32), 1)
    slf = (((1.0 - answer_mask)[:, None, :] + sub[None]) > 0).astype(np.float32) * -10000.0
    enc_add = (1.0 - encoder_mask) * -10000.0  # [B, SE]

    shared = {}
    for l in range(n_layers):
        lp = p["layers"][l]
        for a, pre in enumerate(("slf", "enc")):
            sp = lp[pre]
            for r in range(TP):
                fs = slice(r * LF, (r + 1) * LF)
                wq = _wT(_np(sp["wq"]))[:, :, fs]
                wk = _wT(_np(sp["wk"]))[:, :, fs]
                wv = _wT(_np(sp["wv"]))[:, :, fs]
                shared[(f"wqkv_{l}_{a}", r)] = _bf16(
                    np.concatenate([wq, wk, wv], axis=2))
                woT = np.ascontiguousarray(
                    np.asarray(_np(sp["wo"]), np.float32).T)[fs]  # [LF, H]
                shared[(f"wo_{l}_{a}", r)] = _bf16(np.ascontiguousarray(
                    woT.reshape(LF // 128, 128, H).transpose(1, 0, 2)))
                bq = np.asarray(_np(sp["bq"]), np.float32)[fs] * 0.125
                bk = np.asarray(_np(sp["bk"]), np.float32)[fs]
                bvc = np.concatenate([
                    _pvec(bq), _pvec(bk), _pvec(_np(sp["bo"])),
                    _pvec(_np(sp["ln_g"])), _pvec(_np(sp["ln_b"]))], axis=1)
                shared[(f"bvec_{l}_{a}", r)] = np.ascontiguousarray(bvc)
                shared[(f"bv_{l}_{a}", r)] = _bf16(
                    np.asarray(_np(sp["bv"]), np.float32)[fs][None])
        fp = lp["ffn"]
        for r in range(TP):
            ffs = slice(r * LFF, (r + 1) * LFF)
            shared[(f"w1_{l}", r)] = _bf16(_wT(_np(fp["w1"]))[:, :, ffs])
            w2T = np.ascontiguousarray(
                np.asarray(_np(fp["w2"]), np.float32).T)[ffs]  # [LFF, H]
            shared[(f"w2_{l}", r)] = _bf16(np.ascontiguousarray(
                w2T.reshape(FFC, 128, H).transpose(1, 0, 2)))
            bfc = np.concatenate([
                _pvec(np.asarray(_np(fp["b1"]), np.float32)[ffs]),
                _pvec(_np(fp["b2"])), _pvec(_np(fp["ln_g"])),
                _pvec(_np(fp["ln_b"]))], axis=1)
            shared[(f"bffn_{l}", r)] = np.ascontiguousarray(bfc)

    cls_w_t = _bf16(_wT(_np(p["cls_w"])))
    cls_bvec_t = np.ascontiguousarray(np.concatenate([
        _pvec(_np(p["cls_b"])), _pvec(_np(p["cls_ln_g"])),
        _pvec(_np(p["cls_ln_b"]))], axis=1))
    cls_bias = np.asarray(_np(p["cls_bias"]), np.float32)
    emb_sh, bias_sh = {}, {}
    for r in range(TP):
        shp = np.zeros((VPAD, H), np.float32)
        shp[:VSH] = word_emb[r * VSH:(r + 1) * VSH]
        embT = np.ascontiguousarray(shp.T)  # [H, VPAD]
        emb_sh[r] = _bf16(np.ascontiguousarray(
            embT.reshape(HC, 128, VPAD).transpose(1, 0, 2)))
        bsl = np.zeros((VPAD,), np.float32)
        bsl[:VSH] = cls_bias[r * VSH:(r + 1) * VSH]
        bias_sh[r] = np.ascontiguousarray(
            np.broadcast_to(bsl, (128, VPAD))).astype(np.float32)

    in_maps = []
    for b in range(B):
        x0T = _fmaj(x0[b])
        encT = _fmaj(encoder_outs[b])
        mT = slf[b].T  # [k, q]
        maskT = np.ascontiguousarray(
            mT.reshape(KC, 128, S).transpose(1, 0, 2)).astype(np.float32)
        encm = np.ascontiguousarray(
            enc_add[b].reshape(KC, 128).T).astype(np.float32)
        for r in range(TP):
            m = {"x0T": x0T, "x0Tb": _bf16(x0T), "encTb": _bf16(encT),
                 "maskT": maskT, "encmask": encm,
                 "cls_w": cls_w_t, "cls_bvec": cls_bvec_t,
                 "emb": emb_sh[r], "bias_rep": bias_sh[r]}
            for (name, rr), arr in shared.items():
                if rr == r:
                    m[name] = arr
            in_maps.append(m)
    return in_maps


def assemble(results):
    out = np.empty((B, S, V), np.float32)
    for b in range(B):
        for r in range(TP):
            out[b, :, r * VSH:(r + 1) * VSH] = \
                results[2 * b + r]["logits"][:, :VSH]
    return out


class SpmdRunner:
    """Direct PJRT runner for bass kernels under axon — mirrors
    bass2jax.run_bass_via_pjrt but returns a reusable jitted callable."""

    def __init__(self, nc, n_cores):
        import jax
        import jax.numpy as jnp
        from jax.experimental.shard_map import shard_map
        from jax.sharding import Mesh, PartitionSpec
        import concourse.mybir as mybir
        from concourse import bass2jax

        bass2jax.install_neuronx_cc_hook()
        self.nc = nc
        self.n_cores = n_cores
        self._jax = jax
        self._P = PartitionSpec
        partition_name = (nc.partition_id_tensor.name
                          if nc.partition_id_tensor else None)
        in_names, out_names, out_avals, zero_outs = [], [], [], []
        for alloc in nc.m.functions[0].allocations:
            if not isinstance(alloc, mybir.MemoryLocationSet):
                continue
            name = alloc.memorylocations[0].name
            if alloc.kind == "ExternalInput":
                if name != partition_name:
                    in_names.append(name)
            elif alloc.kind == "ExternalOutput":
                out_names.append(name)
                shape = tuple(alloc.tensor_shape)
                dtype = mybir.dt.np(alloc.dtype)
                out_avals.append(jax.core.ShapedArray(shape, dtype))
                zero_outs.append((shape, dtype))
        self.in_names = list(in_names)
        self.out_names = out_names
        self.out_avals = out_avals
        n_params = len(in_names)
        n_outs = len(out_names)
        all_in_names = list(in_names) + list(out_names)
        if partition_name is not None:
            all_in_names.append(partition_name)

        def _body(*args):
            operands = list(args)
            if partition_name is not None:
                operands.append(bass2jax.partition_id_tensor())
            outs = bass2jax._bass_exec_p.bind(
                *operands,
                out_avals=tuple(out_avals),
                in_names=tuple(all_in_names),
                out_names=tuple(out_names),
                lowering_input_output_aliases=(),
                sim_require_finite=True,
                sim_require_nnan=True,
                nc=nc,
            )
            return tuple(outs)

        devices = jax.devices()[:n_cores]
        assert len(devices) == n_cores
        self.mesh = Mesh(np.asarray(devices), ("core",))
        in_specs = (PartitionSpec("core"),) * (n_params + n_outs)
        out_specs = (PartitionSpec("core"),) * n_outs
        donate = tuple(range(n_params, n_params + n_outs))
        self.sharded = jax.jit(
            shard_map(_body, mesh=self.mesh, in_specs=in_specs,
                      out_specs=out_specs, check_rep=False),
            donate_argnums=donate, keep_unused=True,
        )
        zshapes = [(n_cores * s[0], *s[1:]) for s, d in zero_outs]
        zdtypes = [d for s, d in zero_outs]
        sharding = jax.sharding.NamedSharding(self.mesh, PartitionSpec("core"))

        def _mkzeros():
            return tuple(jnp.zeros(s, d) for s, d in zip(zshapes, zdtypes))

        self.mkzeros = jax.jit(_mkzeros, out_shardings=(sharding,) * n_outs)

    def put_inputs(self, in_maps):
        assert len(in_maps) == self.n_cores
        jax = self._jax
        sharding = jax.sharding.NamedSharding(self.mesh, self._P("core"))
        dev_in = []
        for name in self.in_names:
            concat = np.concatenate(
                [np.asarray(in_maps[c][name]) for c in range(self.n_cores)],
                axis=0)
            dev_in.append(jax.device_put(concat, sharding))
        return dev_in

    def run(self, dev_in):
        return self.sharded(*dev_in, *self.mkzeros())

    def fetch(self, outs):
        res = []
        np_outs = [np.asarray(o) for o in outs]
        for c in range(self.n_cores):
            d = {}
            for i, name in enumerate(self.out_names):
                shape = self.out_avals[i].shape
                d[name] = np_outs[i].reshape(self.n_cores, *shape)[c]
            res.append(d)
        return res


def get_runner(n_layers=L):
    key = n_layers
    if key not in _RUNNER_CACHE:
        nc = build_nc(n_layers)
        _RUNNER_CACHE[key] = SpmdRunner(nc, N_CORES)
    return _RUNNER_CACHE[key]


def run_on_device(in_maps, n_layers=L):
    import jax
    runner = get_runner(n_layers)
    dev_in = runner.put_inputs(in_maps)
    outs = runner.run(dev_in)
    jax.block_until_ready(outs)
    return runner.fetch(outs)


def kernel(input_ids, encoder_outs, answer_mask, encoder_mask, params):
    in_maps = host_inputs(input_ids, encoder_outs, answer_mask, encoder_mask,
                          params)
    results = run_on_device(in_maps)
    return assemble(results)


# revision 9
# speedup vs baseline: 2.6456x; 2.6456x over previous
"""DecoderBertModel Trainium2 kernel.

Sharding: DP-4 over batch x TP-2 (heads/FFN/vocab split) across 8 cores.
Core c = 2*b + r handles batch element b, TP rank r (6 heads, FFN-1536,
vocab half). Pairwise AllReduce after self-attn out-proj, cross-attn
out-proj, and FFN second matmul. Host does the embedding gather +
embedding LayerNorm + mask expansion + final assembly.

On-device dataflow: all activations feature-major [feature_part, token]:
matmuls chain with zero on-device transposes. Scores are computed
transposed (S_T[k,q]); softmax normalization sums come from ones-vector
matmuls over the k-partition axis; per-token scalars are broadcast
across partitions with K=1 matmuls using bf16 hi/lo splitting for
precision. Matmul operands are bf16 (fp32 PSUM accumulation); the
residual stream, LayerNorm, softmax and logits stay fp32.
"""
import math
import numpy as np

H, NH, DH, FF, L, V, MAXP = 768, 12, 64, 3072, 6, 30522, 512
B, S, SE = 4, 512, 512
EPS = 1e-12
N_CORES = 8
TP = 2
HC = H // 128           # 6 feature chunks
LH = NH // TP           # 6 local heads
LF = LH * DH            # 384 local attn features
LFF = FF // TP          # 1536 local ffn features
FFC = LFF // 128        # 12 ffn chunks
VSH = V // TP           # 15261 local vocab
VPAD = 15360            # padded to 30*512
NVC = VPAD // 512       # 30 vocab chunks
TC = S // 128           # 4 token tiles
KC = S // 128           # 4 key chunks

_RUNNER_CACHE = {}


# ----------------------------------------------------------------------------
# device kernel builder
# ----------------------------------------------------------------------------

def build_nc(n_layers=L):
    import concourse.bacc as bacc
    import concourse.mybir as mybir
    import concourse.tile as tile
    from concourse.bass import ts

    F32 = mybir.dt.float32
    BF16 = mybir.dt.bfloat16
    AF = mybir.ActivationFunctionType
    ALU = mybir.AluOpType

    nc = bacc.Bacc("TRN2", target_bir_lowering=False, debug=False,
                   num_devices=N_CORES)

    def din(name, shape, dt=F32):
        return nc.dram_tensor(name, shape, dt, kind="ExternalInput")

    x0T = din("x0T", [128, HC, S])
    x0Tb = din("x0Tb", [128, HC, S], BF16)
    encTb = din("encTb", [128, HC, SE], BF16)
    maskT = din("maskT", [128, KC, S], BF16)    # self-attn 0/1 mask [k_in, kc, q]
    encmask = din("encmask", [128, KC])         # cross additive per key
    wqkv = [din(f"wqkv_{l}_{a}", [128, HC, 3 * LF], BF16)
            for l in range(n_layers) for a in range(2)]
    wo = [din(f"wo_{l}_{a}", [128, HC, H], BF16)
          for l in range(n_layers) for a in range(2)]
    # bvec cols: 0:3 bq*0.125, 3:6 bk, 6:12 bo, 12:18 ln_g, 18:24 ln_b
    bvec = [din(f"bvec_{l}_{a}", [128, 24]) for l in range(n_layers) for a in range(2)]
    bv_attn = [din(f"bv_{l}_{a}", [1, LF], BF16) for l in range(n_layers) for a in range(2)]
    w1 = [din(f"w1_{l}", [128, HC, LFF], BF16) for l in range(n_layers)]
    w2 = [din(f"w2_{l}", [128, FFC, H], BF16) for l in range(n_layers)]
    # bffn cols: 0:12 b1, 12:18 b2, 18:24 ln_g, 24:30 ln_b
    bffn = [din(f"bffn_{l}", [128, FFC + 3 * HC]) for l in range(n_layers)]
    cls_w = din("cls_w", [128, HC, H], BF16)
    cls_bvec = din("cls_bvec", [128, 3 * HC])   # 0:6 cls_b, 6:12 ln_g, 12:18 ln_b
    emb = din("emb", [128, HC, VPAD], BF16)
    bias_rep = din("bias_rep", [128, VPAD])
    logits = nc.dram_tensor("logits", [S, VPAD], F32, kind="ExternalOutput")

    rg = [[0, 1], [2, 3], [4, 5], [6, 7]]

    with tile.TileContext(nc) as tc:
        with (
            tc.tile_pool(name="singles", bufs=1) as singles,
            tc.tile_pool(name="wat", bufs=4) as wat,
            tc.tile_pool(name="wff", bufs=2) as wff,
            tc.tile_pool(name="wsm", bufs=2) as wsm,
            tc.tile_pool(name="res", bufs=2) as res,
            tc.tile_pool(name="act", bufs=2) as actp,
            tc.tile_pool(name="act1", bufs=1) as act1,
            tc.tile_pool(name="pt", bufs=2) as ptp,
            tc.tile_pool(name="dram", bufs=2, space="DRAM") as dram,
            tc.tile_pool(name="mm", bufs=4, space="PSUM") as pmm,
            tc.tile_pool(name="stat", bufs=2, space="PSUM") as pstat,
            tc.tile_pool(name="aux", bufs=2, space="PSUM") as paux,
        ):
            # --- static tiles
            encT_sb = singles.tile([128, HC, SE], BF16, tag="encT")
            nc.sync.dma_start(encT_sb[:], encTb[:])
            maskT_sb = singles.tile([128, KC, S], BF16, tag="maskT")
            nc.sync.dma_start(maskT_sb[:], maskT[:])
            encm_sb = singles.tile([128, KC], F32, tag="encm")
            nc.sync.dma_start(encm_sb[:], encmask[:])
            ones_col = singles.tile([128, 1], BF16, tag="ones_col")
            nc.vector.memset(ones_col[:], 1.0)
            ones_row = singles.tile([1, 128], BF16, tag="ones_row")
            nc.vector.memset(ones_row[:], 1.0)
            eps_t = singles.tile([1, 1], F32, tag="eps")
            nc.vector.memset(eps_t[:], EPS)

            def bcast_hilo(vec_f32, npart, accum_into=None):
                """Broadcast [1,S] fp32 -> psum [npart,S] via bf16 hi/lo MMs."""
                hi = actp.tile([1, S], BF16, tag="bc_hi")
                lo = actp.tile([1, S], BF16, tag="bc_lo")
                t32 = actp.tile([1, S], F32, tag="bc_t32")
                nc.vector.tensor_copy(hi[:], vec_f32)
                nc.vector.tensor_tensor(t32[:], vec_f32, hi[:], ALU.subtract)
                nc.vector.tensor_copy(lo[:], t32[:])
                if accum_into is None:
                    ps = paux.tile([npart, S], F32, tag="aux", name="bc_ps")
                else:
                    ps = accum_into
                nc.tensor.matmul(ps[:], lhsT=ones_row[:, :npart], rhs=hi[:],
                                 start=True, stop=False)
                nc.tensor.matmul(ps[:], lhsT=ones_row[:, :npart], rhs=lo[:],
                                 start=False, stop=True)
                return ps

            def layer_norm(z, zb_out, g_ap, b_ap):
                """In-place LN of fp32 z [128, HC, S] (feature-major), also
                writes bf16 shadow zb_out. g_ap/b_ap: [128, HC] slices."""
                ps_s1 = pstat.tile([1, S], F32, tag="stat")
                ps_s2 = pstat.tile([1, S], F32, tag="stat")
                for c in range(HC):
                    zb = actp.tile([128, S], BF16, tag="ln_zb")
                    nc.scalar.activation(zb[:], z[:, c, :], AF.Identity)
                    sq = actp.tile([128, S], BF16, tag="ln_sq")
                    nc.gpsimd.tensor_mul(sq[:], zb[:], zb[:])
                    nc.tensor.matmul(ps_s1[:], lhsT=ones_col[:], rhs=zb[:],
                                     start=(c == 0), stop=(c == HC - 1))
                    nc.tensor.matmul(ps_s2[:], lhsT=ones_col[:], rhs=sq[:],
                                     start=(c == 0), stop=(c == HC - 1))
                mean = act1.tile([1, S], F32, tag="ln_mean")
                var = act1.tile([1, S], F32, tag="ln_var")
                rstd = act1.tile([1, S], F32, tag="ln_rstd")
                nc.scalar.mul(mean[:], ps_s1[:], 1.0 / H)
                nc.scalar.mul(var[:], ps_s2[:], 1.0 / H)
                nc.vector.tensor_mul(rstd[:], mean[:], mean[:])
                nc.vector.tensor_sub(var[:], var[:], rstd[:])
                nc.scalar.activation(rstd[:], var[:], AF.Sqrt,
                                     bias=eps_t[:], scale=1.0)
                nc.vector.reciprocal(rstd[:], rstd[:])
                ps_mb = bcast_hilo(mean[:], 128)
                ps_rb = bcast_hilo(rstd[:], 128)
                tmp = actp.tile([128, S], F32, tag="ln_tmp")
                for c in range(HC):
                    nc.vector.tensor_tensor(tmp[:], z[:, c, :], ps_mb[:], ALU.subtract)
                    nc.vector.tensor_tensor(tmp[:], tmp[:], ps_rb[:], ALU.mult)
                    nc.vector.tensor_scalar(z[:, c, :], tmp[:],
                                            g_ap[:, c, None], b_ap[:, c, None],
                                            op0=ALU.mult, op1=ALU.add)
                    nc.gpsimd.tensor_copy(zb_out[:, c, :], z[:, c, :])

            def all_reduce(part_sb, out_sb, dt):
                b_in = dram.tile([128, HC, S], dt, tag="ar_in")
                b_out = dram.tile([128, HC, S], dt, tag="ar_out")
                nc.sync.dma_start(b_in[:], part_sb[:])
                nc.gpsimd.collective_compute(
                    "AllReduce", ALU.add,
                    ins=[b_in[:].opt()], outs=[b_out[:].opt()],
                    replica_groups=rg,
                )
                nc.sync.dma_start(out_sb[:], b_out[:])

            def all_gather_ctx(ctxT, ctxF):
                """AG local ctx [128, 3, S] bf16 -> full ctxF [128, 6, S]."""
                g_in = dram.tile([128, LF // 128, S], BF16, tag="ag_in")
                g_out = dram.tile([256, LF // 128, S], BF16, tag="ag_out")
                nc.sync.dma_start(g_in[:], ctxT[:])
                nc.gpsimd.collective_compute(
                    "AllGather", ALU.bypass,
                    ins=[g_in[:].opt()], outs=[g_out[:].opt()],
                    replica_groups=rg,
                )
                nc.sync.dma_start(ctxF[:, 0:LF // 128, :], g_out[0:128])
                nc.sync.dma_start(ctxF[:, LF // 128:HC, :], g_out[128:256])

            def attn_block(qin, qin_b, kvin_b, kv_len, wqkv_t, wo_t, bvec_t,
                           bv_t, is_self):
                """qin: fp32 resid [128,HC,S]; qin_b/kvin_b: bf16 shadows.
                Returns (new fp32 resid tile, new bf16 shadow tile)."""
                kcn = kv_len // 128
                wq_sb = wat.tile([128, HC, LF], BF16, tag="wat")
                nc.sync.dma_start(wq_sb[:], wqkv_t[:, :, 0:LF])
                wk_sb = wat.tile([128, HC, LF], BF16, tag="wat")
                nc.sync.dma_start(wk_sb[:], wqkv_t[:, :, LF:2 * LF])
                wv_sb = wat.tile([128, HC, LF], BF16, tag="wat")
                nc.sync.dma_start(wv_sb[:], wqkv_t[:, :, 2 * LF:3 * LF])
                wo_h = []
                for i in range(2):
                    wo_sb = wat.tile([128, HC // 2, H], BF16, tag="wat",
                                     name=f"wo_sb{i}")
                    nc.sync.dma_start(
                        wo_sb[:], wo_t[:, i * (HC // 2):(i + 1) * (HC // 2), :])
                    wo_h.append(wo_sb)
                bv_sb = wsm.tile([128, 24], F32, tag="bvec")
                nc.sync.dma_start(bv_sb[:], bvec_t[:])
                bvv_sb = wsm.tile([1, LF], BF16, tag="bv_attn")
                nc.sync.dma_start(bvv_sb[:], bv_t[:])

                qT = act1.tile([128, LF // 128, S], BF16, tag="qT")
                kT = act1.tile([128, LF // 128, SE], BF16, tag="kT")
                vtm = act1.tile([128, KC, LF], BF16, tag="vtm")
                for f in range(LF // 128):
                    pq = pmm.tile([128, S], F32, tag="mm")
                    for c in range(HC):
                        nc.tensor.matmul(pq[:], lhsT=wq_sb[:, c, ts(f, 128)],
                                         rhs=qin_b[:, c, :],
                                         start=(c == 0), stop=(c == HC - 1))
                    nc.scalar.activation(qT[:, f, :], pq[:], AF.Identity,
                                         bias=bv_sb[:, f, None], scale=0.125)
                for f in range(LF // 128):
                    pk = pmm.tile([128, kv_len], F32, tag="mm")
                    for c in range(HC):
                        nc.tensor.matmul(pk[:], lhsT=wk_sb[:, c, ts(f, 128)],
                                         rhs=kvin_b[:, c, :],
                                         start=(c == 0), stop=(c == HC - 1))
                    nc.scalar.activation(kT[:, f, :kv_len], pk[:], AF.Identity,
                                         bias=bv_sb[:, 3 + f, None], scale=1.0)
                for t in range(kcn):
                    pv = pmm.tile([128, LF], F32, tag="mm")
                    for c in range(HC):
                        nc.tensor.matmul(pv[:], lhsT=kvin_b[:, c, ts(t, 128)],
                                         rhs=wv_sb[:, c, :],
                                         start=(c == 0), stop=False)
                    nc.tensor.matmul(pv[:], lhsT=ones_row[:], rhs=bvv_sb[:],
                                     start=False, stop=True)
                    nc.scalar.activation(vtm[:, t, :], pv[:], AF.Identity)

                ctxT = act1.tile([128, LF // 128, S], BF16, tag="ctxT")
                for h in range(LH):
                    fchunk, off = h // 2, (h % 2) * DH
                    pT = ptp.tile([128, kcn, S], BF16, tag="pT")
                    for kc in range(kcn):
                        psc = pmm.tile([128, S], F32, tag="mm")
                        nc.tensor.matmul(
                            psc[:],
                            lhsT=kT[off:off + DH, fchunk, ts(kc, 128)],
                            rhs=qT[off:off + DH, fchunk, :],
                            start=True, stop=True)
                        if is_self:
                            nc.scalar.activation(pT[:, kc, :], psc[:], AF.Exp)
                            nc.gpsimd.tensor_mul(pT[:, kc, :], pT[:, kc, :],
                                                 maskT_sb[:, kc, :])
                        else:
                            nc.scalar.activation(pT[:, kc, :], psc[:], AF.Exp,
                                                 bias=encm_sb[:, kc, None],
                                                 scale=1.0)
                    ps_sum = pstat.tile([1, S], F32, tag="stat")
                    for kc in range(kcn):
                        nc.tensor.matmul(ps_sum[:], lhsT=ones_col[:],
                                         rhs=pT[:, kc, :],
                                         start=(kc == 0), stop=(kc == kcn - 1))
                    rcp = act1.tile([1, S], F32, tag="rcp")
                    nc.vector.reciprocal(rcp[:], ps_sum[:])
                    ps_rcpb = bcast_hilo(rcp[:], DH)
                    rcpb_sb = actp.tile([DH, S], F32, tag="rcpb_sb")
                    nc.scalar.activation(rcpb_sb[:], ps_rcpb[:], AF.Identity)
                    ps_ctx = paux.tile([DH, S], F32, tag="aux")
                    for kc in range(kcn):
                        nc.tensor.matmul(ps_ctx[:],
                                         lhsT=vtm[:, kc, ts(h, DH)],
                                         rhs=pT[:, kc, :],
                                         start=(kc == 0), stop=(kc == kcn - 1))
                    nc.vector.tensor_mul(ctxT[off:off + DH, fchunk, :],
                                         ps_ctx[:], rcpb_sb[:])

                ctxF = act1.tile([128, HC, S], BF16, tag="ctxF")
                all_gather_ctx(ctxT, ctxF)
                z = res.tile([128, HC, S], F32, tag="res")
                for c in range(HC):
                    po = pmm.tile([128, S], F32, tag="mm")
                    for f in range(HC):
                        wo_sb, fo = (wo_h[0], f) if f < HC // 2 else (wo_h[1], f - HC // 2)
                        nc.tensor.matmul(po[:], lhsT=wo_sb[:, fo, ts(c, 128)],
                                         rhs=ctxF[:, f, :],
                                         start=(f == 0), stop=(f == HC - 1))
                    nc.scalar.activation(z[:, c, :], po[:], AF.Identity,
                                         bias=bv_sb[:, 6 + c, None], scale=1.0)
                    nc.gpsimd.tensor_add(z[:, c, :], z[:, c, :], qin[:, c, :])
                zb = res.tile([128, HC, S], BF16, tag="resb")
                layer_norm(z, zb, bv_sb[:, 12:18], bv_sb[:, 18:24])
                return z, zb

            def ffn_block(a, a_b, w1_t, w2_t, bffn_t):
                w1h = []
                for i in range(2):
                    w1_sb = wff.tile([128, HC, LFF // 2], BF16, tag="wff",
                                     name=f"w1_sb{i}")
                    nc.sync.dma_start(
                        w1_sb[:], w1_t[:, :, i * (LFF // 2):(i + 1) * (LFF // 2)])
                    w1h.append(w1_sb)
                bf_sb = wsm.tile([128, FFC + 3 * HC], F32, tag="bffn")
                nc.sync.dma_start(bf_sb[:], bffn_t[:])
                hT = act1.tile([128, FFC, S], BF16, tag="hT")
                for fc in range(FFC):
                    w1_sb, fo = (w1h[0], fc) if fc < FFC // 2 else (w1h[1], fc - FFC // 2)
                    ph = pmm.tile([128, S], F32, tag="mm")
                    for c in range(HC):
                        nc.tensor.matmul(ph[:], lhsT=w1_sb[:, c, ts(fo, 128)],
                                         rhs=a_b[:, c, :],
                                         start=(c == 0), stop=(c == HC - 1))
                    nc.scalar.activation(hT[:, fc, :], ph[:], AF.Gelu,
                                         bias=bf_sb[:, fc, None], scale=1.0)
                w2h = []
                for i in range(2):
                    w2_sb = wff.tile([128, FFC // 2, H], BF16, tag="wff",
                                     name=f"w2_sb{i}")
                    nc.sync.dma_start(
                        w2_sb[:], w2_t[:, i * (FFC // 2):(i + 1) * (FFC // 2), :])
                    w2h.append(w2_sb)
                zpart = act1.tile([128, HC, S], BF16, tag="zpart")
                for c in range(HC):
                    pz = pmm.tile([128, S], F32, tag="mm")
                    for fc in range(FFC):
                        w2_sb, fo = (w2h[0], fc) if fc < FFC // 2 else (w2h[1], fc - FFC // 2)
                        nc.tensor.matmul(pz[:], lhsT=w2_sb[:, fo, ts(c, 128)],
                                         rhs=hT[:, fc, :],
                                         start=(fc == 0), stop=(fc == FFC - 1))
                    nc.scalar.activation(zpart[:, c, :], pz[:], AF.Identity)
                zar = act1.tile([128, HC, S], BF16, tag="zar")
                all_reduce(zpart, zar, BF16)
                z = res.tile([128, HC, S], F32, tag="res")
                for c in range(HC):
                    nc.gpsimd.tensor_scalar(z[:, c, :], zar[:, c, :],
                                            bf_sb[:, FFC + c, None], None,
                                            op0=ALU.add)
                    nc.gpsimd.tensor_add(z[:, c, :], z[:, c, :], a[:, c, :])
                zb = res.tile([128, HC, S], BF16, tag="resb")
                layer_norm(z, zb, bf_sb[:, FFC + HC:FFC + 2 * HC],
                           bf_sb[:, FFC + 2 * HC:FFC + 3 * HC])
                return z, zb

            # ---- load embeddings into residual stream
            x = res.tile([128, HC, S], F32, tag="res")
            nc.sync.dma_start(x[:], x0T[:])
            xb = res.tile([128, HC, S], BF16, tag="resb")
            nc.sync.dma_start(xb[:], x0Tb[:])

            # ---- transformer body
            for l in range(n_layers):
                a, ab = attn_block(x, xb, xb, S, wqkv[2 * l], wo[2 * l],
                                   bvec[2 * l], bv_attn[2 * l], is_self=True)
                a2, a2b = attn_block(a, ab, encT_sb, SE, wqkv[2 * l + 1],
                                     wo[2 * l + 1], bvec[2 * l + 1],
                                     bv_attn[2 * l + 1], is_self=False)
                x, xb = ffn_block(a2, a2b, w1[l], w2[l], bffn[l])

            # ---- classifier head: t = LN(gelu(x @ cls_w.T + cls_b))
            cw_sb = wff.tile([128, HC, H], BF16, tag="wff")
            nc.sync.dma_start(cw_sb[:], cls_w[:])
            cb_sb = wsm.tile([128, 3 * HC], F32, tag="bffn")
            nc.sync.dma_start(cb_sb[:], cls_bvec[:])
            g = res.tile([128, HC, S], F32, tag="res")
            for c in range(HC):
                pg = pmm.tile([128, S], F32, tag="mm")
                for c2 in range(HC):
                    nc.tensor.matmul(pg[:], lhsT=cw_sb[:, c2, ts(c, 128)],
                                     rhs=xb[:, c2, :],
                                     start=(c2 == 0), stop=(c2 == HC - 1))
                nc.scalar.activation(g[:, c, :], pg[:], AF.Gelu,
                                     bias=cb_sb[:, c, None], scale=1.0)
            tb = res.tile([128, HC, S], BF16, tag="resb")
            layer_norm(g, tb, cb_sb[:, HC:2 * HC], cb_sb[:, 2 * HC:3 * HC])

            # ---- vocab projection: logits[t, v] = t.T @ emb + bias
            for vc in range(NVC):
                ech = wff.tile([128, HC, 512], BF16, tag="wff")
                nc.sync.dma_start(ech[:], emb[:, :, ts(vc, 512)])
                bch = wsm.tile([128, 512], F32, tag="bias_ch")
                nc.sync.dma_start(bch[:], bias_rep[:, ts(vc, 512)])
                for t in range(TC):
                    pl = pmm.tile([128, 512], F32, tag="mm")
                    for c in range(HC):
                        nc.tensor.matmul(pl[:], lhsT=tb[:, c, ts(t, 128)],
                                         rhs=ech[:, c, :],
                                         start=(c == 0), stop=(c == HC - 1))
                    lsb = actp.tile([128, 512], F32, tag="lsb")
                    nc.vector.tensor_add(lsb[:], pl[:], bch[:])
                    nc.sync.dma_start(logits[ts(t, 128), ts(vc, 512)], lsb[:])

    nc.finalize()
    return nc


# ----------------------------------------------------------------------------
# host-side preprocessing
# ----------------------------------------------------------------------------

def _np(x):
    return np.asarray(x)


def _bf16(x):
    import ml_dtypes
    return np.asarray(x, dtype=np.float32).astype(ml_dtypes.bfloat16)


def _ln_host(x, g, b):
    m = x.mean(axis=-1, keepdims=True)
    v = ((x - m) ** 2).mean(axis=-1, keepdims=True)
    return (x - m) / np.sqrt(v + EPS) * g + b


def _fmaj(x):
    """[S, H] -> [128, H//128, S] feature-major chunks."""
    s, h = x.shape
    return np.ascontiguousarray(
        x.T.reshape(h // 128, 128, s).transpose(1, 0, 2)).astype(np.float32)


def _wT(w):
    """torch Linear weight [out, in] -> [128, in//128, out] lhsT layout."""
    win = np.ascontiguousarray(np.asarray(w, dtype=np.float32).T)  # [in, out]
    i, o = win.shape
    return np.ascontiguousarray(
        win.reshape(i // 128, 128, o).transpose(1, 0, 2))


def _pvec(v):
    """[F] -> [128, F//128] per-partition layout."""
    v = np.asarray(v, dtype=np.float32)
    f = v.shape[0]
    return np.ascontiguousarray(v.reshape(f // 128, 128).T)


def host_inputs(input_ids, encoder_outs, answer_mask, encoder_mask, params,
                n_layers=L):
    p = params
    input_ids = _np(input_ids)
    encoder_outs = np.asarray(_np(encoder_outs), dtype=np.float32)
    answer_mask = np.asarray(_np(answer_mask), dtype=np.float32)
    encoder_mask = np.asarray(_np(encoder_mask), dtype=np.float32)

    word_emb = np.asarray(_np(p["word_emb"]), dtype=np.float32)
    pos_emb = np.asarray(_np(p["pos_emb"]), dtype=np.float32)
    emb = word_emb[input_ids] + pos_emb[:S][None]
    x0 = _ln_host(emb.astype(np.float64),
                  np.asarray(_np(p["emb_ln_g"]), dtype=np.float64),
                  np.asarray(_np(p["emb_ln_b"]), dtype=np.float64)
                  ).astype(np.float32)

    sub = np.triu(np.ones((S, S), np.float32), 1)
    slf = (((1.0 - answer_mask)[:, None, :] + sub[None]) > 0).astype(np.float32) * -10000.0
    enc_add = (1.0 - encoder_mask) * -10000.0  # [B, SE]

    shared = {}
    for l in range(n_layers):
        lp = p["layers"][l]
        for a, pre in enumerate(("slf", "enc")):
            sp = lp[pre]
            for r in range(TP):
                fs = slice(r * LF, (r + 1) * LF)
                wq = _wT(_np(sp["wq"]))[:, :, fs]
                wk = _wT(_np(sp["wk"]))[:, :, fs]
                wv = _wT(_np(sp["wv"]))[:, :, fs]
                shared[(f"wqkv_{l}_{a}", r)] = _bf16(
                    np.concatenate([wq, wk, wv], axis=2))
                shared[(f"wo_{l}_{a}", r)] = _bf16(_wT(_np(sp["wo"])))
                bq = np.asarray(_np(sp["bq"]), np.float32)[fs] * 0.125
                bk = np.asarray(_np(sp["bk"]), np.float32)[fs]
                bvc = np.concatenate([
                    _pvec(bq), _pvec(bk), _pvec(_np(sp["bo"])),
                    _pvec(_np(sp["ln_g"])), _pvec(_np(sp["ln_b"]))], axis=1)
                shared[(f"bvec_{l}_{a}", r)] = np.ascontiguousarray(bvc)
                shared[(f"bv_{l}_{a}", r)] = _bf16(
                    np.asarray(_np(sp["bv"]), np.float32)[fs][None])
        fp = lp["ffn"]
        for r in range(TP):
            ffs = slice(r * LFF, (r + 1) * LFF)
            shared[(f"w1_{l}", r)] = _bf16(_wT(_np(fp["w1"]))[:, :, ffs])
            w2T = np.ascontiguousarray(
                np.asarray(_np(fp["w2"]), np.float32).T)[ffs]  # [LFF, H]
            shared[(f"w2_{l}", r)] = _bf16(np.ascontiguousarray(
                w2T.reshape(FFC, 128, H).transpose(1, 0, 2)))
            bfc = np.concatenate([
                _pvec(np.asarray(_np(fp["b1"]), np.float32)[ffs]),
                _pvec(_np(fp["b2"])), _pvec(_np(fp["ln_g"])),
                _pvec(_np(fp["ln_b"]))], axis=1)
            shared[(f"bffn_{l}", r)] = np.ascontiguousarray(bfc)

    cls_w_t = _bf16(_wT(_np(p["cls_w"])))
    cls_bvec_t = np.ascontiguousarray(np.concatenate([
        _pvec(_np(p["cls_b"])), _pvec(_np(p["cls_ln_g"])),
        _pvec(_np(p["cls_ln_b"]))], axis=1))
    cls_bias = np.asarray(_np(p["cls_bias"]), np.float32)
    emb_sh, bias_sh = {}, {}
    for r in range(TP):
        shp = np.zeros((VPAD, H), np.float32)
        shp[:VSH] = word_emb[r * VSH:(r + 1) * VSH]
        embT = np.ascontiguousarray(shp.T)  # [H, VPAD]
        emb_sh[r] = _bf16(np.ascontiguousarray(
            embT.reshape(HC, 128, VPAD).transpose(1, 0, 2)))
        bsl = np.zeros((VPAD,), np.float32)
        bsl[:VSH] = cls_bias[r * VSH:(r + 1) * VSH]
        bias_sh[r] = np.ascontiguousarray(
            np.broadcast_to(bsl, (128, VPAD))).astype(np.float32)

    in_maps = []
    for b in range(B):
        x0T = _fmaj(x0[b])
        encT = _fmaj(encoder_outs[b])
        mT = (slf[b].T > -5000.0).astype(np.float32)  # [k, q] 0/1 keep-mask
        maskT = _bf16(np.ascontiguousarray(
            mT.reshape(KC, 128, S).transpose(1, 0, 2)))
        encm = np.ascontiguousarray(
            enc_add[b].reshape(KC, 128).T).astype(np.float32)
        for r in range(TP):
            m = {"x0T": x0T, "x0Tb": _bf16(x0T), "encTb": _bf16(encT),
                 "maskT": maskT, "encmask": encm,
                 "cls_w": cls_w_t, "cls_bvec": cls_bvec_t,
                 "emb": emb_sh[r], "bias_rep": bias_sh[r]}
            for (name, rr), arr in shared.items():
                if rr == r:
                    m[name] = arr
            in_maps.append(m)
    return in_maps


def assemble(results):
    out = np.empty((B, S, V), np.float32)
    for b in range(B):
        for r in range(TP):
            out[b, :, r * VSH:(r + 1) * VSH] = \
                results[2 * b + r]["logits"][:, :VSH]
    return out


class SpmdRunner:
    """Direct PJRT runner for bass kernels under axon — mirrors
    bass2jax.run_bass_via_pjrt but returns a reusable jitted callable."""

    def __init__(self, nc, n_cores):
        import jax
        import jax.numpy as jnp
        from jax.experimental.shard_map import shard_map
        from jax.sharding import Mesh, PartitionSpec
        import concourse.mybir as mybir
        from concourse import bass2jax

        bass2jax.install_neuronx_cc_hook()
        self.nc = nc
        self.n_cores = n_cores
        self._jax = jax
        self._P = PartitionSpec
        partition_name = (nc.partition_id_tensor.name
                          if nc.partition_id_tensor else None)
        in_names, out_names, out_avals, zero_outs = [], [], [], []
        for alloc in nc.m.functions[0].allocations:
            if not isinstance(alloc, mybir.MemoryLocationSet):
                continue
            name = alloc.memorylocations[0].name
            if alloc.kind == "ExternalInput":
                if name != partition_name:
                    in_names.append(name)
            elif alloc.kind == "ExternalOutput":
                out_names.append(name)
                shape = tuple(alloc.tensor_shape)
                dtype = mybir.dt.np(alloc.dtype)
                out_avals.append(jax.core.ShapedArray(shape, dtype))
                zero_outs.append((shape, dtype))
        self.in_names = list(in_names)
        self.out_names = out_names
        self.out_avals = out_avals
        n_params = len(in_names)
        n_outs = len(out_names)
        all_in_names = list(in_names) + list(out_names)
        if partition_name is not None:
            all_in_names.append(partition_name)

        def _body(*args):
            operands = list(args)
            if partition_name is not None:
                operands.append(bass2jax.partition_id_tensor())
            outs = bass2jax._bass_exec_p.bind(
                *operands,
                out_avals=tuple(out_avals),
                in_names=tuple(all_in_names),
                out_names=tuple(out_names),
                lowering_input_output_aliases=(),
                sim_require_finite=True,
                sim_require_nnan=True,
                nc=nc,
            )
            return tuple(outs)

        devices = jax.devices()[:n_cores]
        assert len(devices) == n_cores
        self.mesh = Mesh(np.asarray(devices), ("core",))
        in_specs = (PartitionSpec("core"),) * (n_params + n_outs)
        out_specs = (PartitionSpec("core"),) * n_outs
        donate = tuple(range(n_params, n_params + n_outs))
        self.sharded = jax.jit(
            shard_map(_body, mesh=self.mesh, in_specs=in_specs,
                      out_specs=out_specs, check_rep=False),
            donate_argnums=donate, keep_unused=True,
        )
        zshapes = [(n_cores * s[0], *s[1:]) for s, d in zero_outs]
        zdtypes = [d for s, d in zero_outs]
        sharding = jax.sharding.NamedSharding(self.mesh, PartitionSpec("core"))

        def _mkzeros():
            return tuple(jnp.zeros(s, d) for s, d in zip(zshapes, zdtypes))

        self.mkzeros = jax.jit(_mkzeros, out_shardings=(sharding,) * n_outs)

    def put_inputs(self, in_maps):
        assert len(in_maps) == self.n_cores
        jax = self._jax
        sharding = jax.sharding.NamedSharding(self.mesh, self._P("core"))
        dev_in = []
        for name in self.in_names:
            concat = np.concatenate(
                [np.asarray(in_maps[c][name]) for c in range(self.n_cores)],
                axis=0)
            dev_in.append(jax.device_put(concat, sharding))
        return dev_in

    def run(self, dev_in):
        return self.sharded(*dev_in, *self.mkzeros())

    def fetch(self, outs):
        res = []
        np_outs = [np.asarray(o) for o in outs]
        for c in range(self.n_cores):
            d = {}
            for i, name in enumerate(self.out_names):
                shape = self.out_avals[i].shape
                d[name] = np_outs[i].reshape(self.n_cores, *shape)[c]
            res.append(d)
        return res


def get_runner(n_layers=L):
    key = n_layers
    if key not in _RUNNER_CACHE:
        nc = build_nc(n_layers)
        _RUNNER_CACHE[key] = SpmdRunner(nc, N_CORES)
    return _RUNNER_CACHE[key]


def run_on_device(in_maps, n_layers=L):
    import jax
    runner = get_runner(n_layers)
    dev_in = runner.put_inputs(in_maps)
    outs = runner.run(dev_in)
    jax.block_until_ready(outs)
    return runner.fetch(outs)


def kernel(input_ids, encoder_outs, answer_mask, encoder_mask, params):
    in_maps = host_inputs(input_ids, encoder_outs, answer_mask, encoder_mask,
                          params)
    results = run_on_device(in_maps)
    return assemble(results)
